# revision 55
# baseline (speedup 1.0000x reference)
"""Trainium2 Bass kernel for the fc-score attention module (Synthesizer-style).

Math per (batch b, head h), with q,k,v = per-head slices of x @ w_qkv.T:
    a   = (q*scale) @ k.T @ w_fc.T + b_fc          # re-associated: q @ (k.T @ w_fc.T)
    t   = LayerNorm(a) * gamma + beta
    e   = exp(t)                                    # softmax numerator (LN bounds => no max sub)
    S   = rowsum(e)
    y   = (v / S).T @ e                             # softmax denominator folded into v
    out = (y over kw) @ w_fc2.T + b_fc2             # via y transpose + matmul

Key algebraic facts used (all exact):
  - (q@k.T)@w_fc.T == q@(k.T@w_fc.T): d=64 inner dim cuts the dominant matmul ~8x.
  - row mean of a:   mu = q @ rowmean(kf)
  - row sumsq of a:  sq[n] = q[n] @ (kf@kf.T) @ q[n].T
    so LN stats never require touching the [N,KW] matrix with the vector engine.
  - LN output is bounded by sqrt(KW), so exp() cannot overflow in fp32.

Sharding: batch (B=8) across the 8 NeuronCores; per-core all 12 heads are
processed in 6 adjacent pairs so that most ops use the full 128 partitions.

Scheduling: the softmax exp stream on ScalarE (~96 x 1.5us) is the serial
backbone of the kernel.  r/b for each pair is computed with
rsqrt(x) = exp(-0.5*ln(x)) so that Exp/Ln share one activation-table set and
pair 0's exp can start as soon as ITS stats are done (no global barrier).
Emission interleaves pair p's softmax/AV (B2) with pair p+1's LN-statistics
pipeline (B1) chunk by chunk so TensorE always has dense work queued -> the
HAM clock gate stays at full rate (the v1 kernel spent 177us at half clock).
"""

import os

os.environ.setdefault("MYCRO_LOCAL_CACHE", "1")

import numpy as np
import ml_dtypes

import concourse.bass as bass
import concourse.mybir as mybir
import concourse.tile as tile
from concourse import bacc
from concourse.bass_utils import run_bass_kernel_spmd
from concourse.masks import make_identity

H = 12
EPS = 1e-5
B, N, C, KW = 8, 1024, 768, 1024
D = C // H  # 64
SCALE = D ** -0.5
NP_ = 128          # partitions
NCH = N // NP_     # 8 chunks of sequence
CCH = C // NP_     # 6 chunks of channels
KCH = KW // NP_    # 8 chunks of kw
NPAIR = H // 2     # 6 head pairs

F32 = mybir.dt.float32
BF16 = mybir.dt.bfloat16

_CACHE = {}
LAST_RESULT = None  # test harness can inspect exec_time_ns etc.


def _build(aug: bool, gb: bool, aug2: bool):
    """Build the per-core Bass program.

    aug:  b_fc is nonzero (extra broadcast-add before exp + stats correction)
    gb:   gamma/beta are nontrivial (explicit LN affine + exp passes)
    aug2: b_fc2 is nonzero (extra K=1 matmul into fc2 accumulation)
    """
    nc = bacc.Bacc("TRN2", target_bir_lowering=False, debug=False, num_devices=8)

    xT_d = nc.dram_tensor("xT", [CCH, NP_, N], BF16, kind="ExternalInput").ap()
    wqT_d = nc.dram_tensor("wqT", [CCH, NP_, C], BF16, kind="ExternalInput").ap()
    wkvT_d = nc.dram_tensor("wkvT", [CCH, NP_, 2 * C], BF16, kind="ExternalInput").ap()
    wfcT_d = nc.dram_tensor("wfcT", [NCH, NP_, KW], BF16, kind="ExternalInput").ap()
    wfc2T_d = nc.dram_tensor("wfc2T", [KCH, NP_, N], BF16, kind="ExternalInput").ap()
    out_d = nc.dram_tensor("out", [NPAIR, NP_, N], F32, kind="ExternalOutput").ap()
    if aug:
        bfc_d = nc.dram_tensor("bfc", [1, KW], F32, kind="ExternalInput").ap()
        bfcc_d = nc.dram_tensor("bfcc", [NP_, KCH], BF16, kind="ExternalInput").ap()
        meanb_d = nc.dram_tensor("cmeanb", [1, 1], F32, kind="ExternalInput").ap()
        sb2_d = nc.dram_tensor("csb2", [1, 1], F32, kind="ExternalInput").ap()
    if gb:
        gam_d = nc.dram_tensor("gam", [1, KW], F32, kind="ExternalInput").ap()
        bet_d = nc.dram_tensor("bet", [1, KW], F32, kind="ExternalInput").ap()
    if aug2:
        bfc2_d = nc.dram_tensor("bfc2", [1, N], BF16, kind="ExternalInput").ap()

    def bcast(ap1xN, parts=NP_):
        # [1, F] dram AP -> partition-broadcast [parts, F]
        return bass.AP(tensor=ap1xN.tensor, offset=ap1xN.offset,
                       ap=[[0, parts]] + list(ap1xN.ap[1:]))

    from contextlib import ExitStack
    with tile.TileContext(nc) as tc, ExitStack() as ctx:
        const = ctx.enter_context(tc.tile_pool(name="const", bufs=1))
        wpool = ctx.enter_context(tc.tile_pool(name="wpool", bufs=1))
        work = ctx.enter_context(tc.tile_pool(name="work", bufs=2))
        epool = ctx.enter_context(tc.tile_pool(name="epool", bufs=5))
        tiny = ctx.enter_context(tc.tile_pool(name="tiny", bufs=8))
        # PSUM budget (8 banks):
        #   "ap"    f32 [128,1024] x2 bufs = 4 banks (score rows: one being
        #           exp'd by ScalarE while PE fills the other -> ACT never
        #           starves, PE never waits on ACT)
        #   "flow"  f32 [128,512]  x2 bufs = 2 banks (B1/B3 transients:
        #           kf/tT halves, sig, stats, transposes, o2 - so the LN
        #           pipeline never competes with the softmax stream)
        #   "hold"  f32 [128,1024] x1 buf  = 2 banks (AV accumulator)
        ppap = ctx.enter_context(tc.tile_pool(name="ppap", bufs=2, space="PSUM"))
        ppflow = ctx.enter_context(tc.tile_pool(name="ppflow", bufs=2, space="PSUM"))
        pphold = ctx.enter_context(tc.tile_pool(name="pphold", bufs=1, space="PSUM"))

        # ---------------- persistent SBUF ----------------
        xT_sb = wpool.tile([NP_, CCH, N], BF16)
        wqT_sb = wpool.tile([NP_, CCH, C], BF16)
        wkvT_sb = wpool.tile([NP_, CCH, 2 * C], BF16)
        wfcT_sb = wpool.tile([NP_, NCH, KW], BF16)
        wfc2T_sb = wpool.tile([NP_, KCH, N], BF16)
        kv_sb = wpool.tile([NP_, NCH, C], BF16)   # k projections only
        v_sb = wpool.tile([NP_, NCH, C], F32)     # v in f32 for gpsimd normalize
        qTa_sb = wpool.tile([NP_, NPAIR, N], BF16)   # [0:64]=even head qT, [64:128]=odd
        kfa_sb = wpool.tile([NP_, NPAIR, KW], BF16)  # same pair layout
        yT_all = wpool.tile([NP_, NPAIR, KW], BF16)  # AV results awaiting fc2
        mu_all = wpool.tile([NP_, NPAIR, 16], F32)   # per pair: cols 0:8 even, 8:16 odd
        sq_all = wpool.tile([NP_, NPAIR, 16], F32)
        r_all = wpool.tile([NP_, NPAIR, 16], F32)
        b_all = wpool.tile([NP_, NPAIR, 16], F32)

        ident = const.tile([NP_, NP_], BF16)
        make_identity(nc, ident)
        ones_c = const.tile([NP_, 1], BF16)
        nc.vector.memset(ones_c, 1.0)
        invkw_c = const.tile([NP_, 1], BF16)
        nc.vector.memset(invkw_c, 1.0 / KW)
        ones_r = const.tile([1, NP_], BF16)
        nc.vector.memset(ones_r, 1.0)
        eps_c = const.tile([NP_, 1], F32)
        nc.vector.memset(eps_c, EPS)
        if aug:
            bfc_bc = const.tile([NP_, KW], F32)
            nc.sync.dma_start(out=bfc_bc, in_=bcast(bfc_d))
            bfcc_sb = const.tile([NP_, KCH], BF16)
            nc.sync.dma_start(out=bfcc_sb, in_=bfcc_d)
            meanb_sb = const.tile([NP_, 1], F32)
            nc.sync.dma_start(out=meanb_sb, in_=bcast(meanb_d))
            sb2_sb = const.tile([NP_, 1], F32)
            nc.sync.dma_start(out=sb2_sb, in_=bcast(sb2_d))
        if gb:
            gam_bc = const.tile([NP_, KW], F32)
            nc.sync.dma_start(out=gam_bc, in_=bcast(gam_d))
            bet_bc = const.tile([NP_, KW], F32)
            nc.sync.dma_start(out=bet_bc, in_=bcast(bet_d))
        if aug2:
            bfc2_sb = const.tile([1, N], BF16)
            nc.sync.dma_start(out=bfc2_sb, in_=bfc2_d)

        # ---------------- input DMAs ----------------
        # Order by first use: the narrow k block needs xT+wkvT, then qT
        # needs wqT, kf needs wfcT; wfc2T is only consumed by the fc2 tail.
        for c in range(CCH):
            nc.sync.dma_start(out=xT_sb[:, c, :], in_=xT_d[c])
            nc.sync.dma_start(out=wkvT_sb[:, c, :], in_=wkvT_d[c])
        for n in range(NCH):
            nc.sync.dma_start(out=wfcT_sb[:, n, :], in_=wfcT_d[n])
        for c in range(CCH):
            nc.sync.dma_start(out=wqT_sb[:, c, :], in_=wqT_d[c])
        for n in range(NCH):
            nc.sync.dma_start(out=wfc2T_sb[:, n, :], in_=wfc2T_d[n])

        mm = nc.tensor.matmul
        cp = nc.vector.tensor_copy

        # HAM warm-up: dummy matmuls during the input-DMA window so the PE
        # clock is at 8/8 when real work starts (zero data deps).
        warm_sb = const.tile([NP_, 512], BF16)
        nc.vector.memset(warm_sb, 0.0)
        wup = ppflow.tile([NP_, 512], F32, tag="flow")
        for _ in range(56):
            mm(wup, warm_sb[:, 0:128], warm_sb, start=True, stop=True)

        # ---------------- stage A ----------------
        def emit_kv_chunk(fs, n, scalar_cp):
            # kv (row layout): out[n*128+p, j] = sum_c xT[c, n*128+p] * wkvT[c, j]
            # k lands in kv_sb (bf16); v lands in v_sb (f32, for the gpsimd
            # normalize).  fs1 straddles the k/v boundary at column 768.
            kvp = ppflow.tile([NP_, 512], F32, tag="flow")
            for c in range(CCH):
                mm(kvp, xT_sb[:, c, n * NP_:(n + 1) * NP_],
                   wkvT_sb[:, c, fs * 512:(fs + 1) * 512],
                   start=(c == 0), stop=(c == CCH - 1))
            if fs == 0:
                if scalar_cp:
                    nc.scalar.copy(out=kv_sb[:, n, 0:512], in_=kvp)
                else:
                    cp(out=kv_sb[:, n, 0:512], in_=kvp)
            elif fs == 1:
                if scalar_cp:
                    nc.scalar.copy(out=kv_sb[:, n, 512:768], in_=kvp[:, 0:256])
                else:
                    cp(out=kv_sb[:, n, 512:768], in_=kvp[:, 0:256])
                cp(out=v_sb[:, n, 0:256], in_=kvp[:, 256:512])
            else:
                cp(out=v_sb[:, n, 256:768], in_=kvp)

        def emit_k_narrow():
            # k columns 0:128 only (pair 0's kf input) so pair 0's whole
            # B1 chain can start ~10us after the xT/wkvT DMAs land, long
            # before the full k projection is done.  Four n-chunks share a
            # PSUM slot; one strided cast evacuates them.
            for n4 in range(2):
                kvp = ppflow.tile([NP_, 512], F32, tag="flow")
                for nn in range(4):
                    n = 4 * n4 + nn
                    for c in range(CCH):
                        mm(kvp[:, nn * NP_:(nn + 1) * NP_],
                           xT_sb[:, c, n * NP_:(n + 1) * NP_],
                           wkvT_sb[:, c, 0:NP_],
                           start=(c == 0), stop=(c == CCH - 1))
                nc.scalar.copy(out=kv_sb[:, 4 * n4:4 * n4 + 4, 0:NP_],
                               in_=kvp.rearrange("p (a b) -> p a b", a=4))

        def emit_k_rest(n):
            # k columns 128:512 for one n-chunk (pairs 1-3 head columns).
            kvp = ppflow.tile([NP_, 384], F32, tag="flow")
            for c in range(CCH):
                mm(kvp, xT_sb[:, c, n * NP_:(n + 1) * NP_],
                   wkvT_sb[:, c, NP_:512],
                   start=(c == 0), stop=(c == CCH - 1))
            # DVE, not ScalarE: these run after B1(0) is emitted, and the
            # scalar queue must stay clear ahead of the first exps.
            cp(out=kv_sb[:, n, NP_:512], in_=kvp)

        def emit_qT(m, scalar_cp):
            # qT (pair layout): out[m*128+p, n] = sum_c wqT[c, m*128+p] * xT[c, n]
            for half in range(2):
                qp = ppflow.tile([NP_, 512], F32, tag="flow")
                for c in range(CCH):
                    lhs = wqT_sb[:, c, m * NP_:(m + 1) * NP_]
                    mm(qp, lhs, xT_sb[:, c, half * 512:(half + 1) * 512],
                       start=(c == 0), stop=(c == CCH - 1))
                if scalar_cp:
                    nc.scalar.copy(out=qTa_sb[:, m, half * 512:(half + 1) * 512],
                                   in_=qp)
                else:
                    cp(out=qTa_sb[:, m, half * 512:(half + 1) * 512], in_=qp)

        # ---------------- B1: per-pair LN statistics, split into chunks ----------------
        def b1_chunks(p):
            """Return a list of thunks; executing all of them emits B1(p)."""
            h0 = 2 * p
            state = {}

            def c_kf0():
                kfp = ppflow.tile([NP_, 512], F32, tag="flow")
                for n in range(NCH):
                    lhs = kv_sb[:, n, h0 * D:h0 * D + NP_]
                    mm(kfp, lhs, wfcT_sb[:, n, 0:512],
                       start=(n == 0), stop=(n == NCH - 1))
                cp(out=kfa_sb[:, p, 0:512], in_=kfp)

            def c_kf1():
                kfp = ppflow.tile([NP_, 512], F32, tag="flow")
                for n in range(NCH):
                    lhs = kv_sb[:, n, h0 * D:h0 * D + NP_]
                    mm(kfp, lhs, wfcT_sb[:, n, 512:1024],
                       start=(n == 0), stop=(n == NCH - 1))
                cp(out=kfa_sb[:, p, 512:1024], in_=kfp)

            def c_tr():
                # kfT via PE transposes; four 128x128 transposes land in one
                # PSUM tile so one DVE copy evacuates all four.
                kfT_sb = work.tile([NP_, KCH, NP_], BF16, tag="kfT")
                state["kfT"] = kfT_sb
                for j4 in range(KCH // 4):
                    trp = ppflow.tile([NP_, 4 * NP_], BF16, tag="flow")
                    for q4 in range(4):
                        jj = 4 * j4 + q4
                        nc.tensor.transpose(
                            trp[:, q4 * NP_:(q4 + 1) * NP_],
                            kfa_sb[:, p, jj * NP_:(jj + 1) * NP_], ident)
                    cp(out=kfT_sb[:, 4 * j4:4 * j4 + 4, :], in_=trp)

            def c_sig():
                kfT_sb = state["kfT"]
                # Sigma (pair block-diagonal): sig = kfT.T @ kfT.  The same
                # loaded stationary also yields kfm = rowmean(kf) as an
                # F=1 matmul against a 1/KW column (keeps it off the DVE).
                sgp = ppflow.tile([NP_, NP_], F32, tag="flow")
                kfmp = ppflow.tile([NP_, 1], F32, tag="flow")
                for j in range(KCH):
                    mm(sgp, kfT_sb[:, j, :], kfT_sb[:, j, :],
                       start=(j == 0), stop=(j == KCH - 1))
                    mm(kfmp, kfT_sb[:, j, :], invkw_c,
                       start=(j == 0), stop=(j == KCH - 1))
                sig_sb = work.tile([NP_, NP_], BF16, tag="sig")
                state["sig"] = sig_sb
                cp(out=sig_sb, in_=sgp)
                kfm_sb = tiny.tile([NP_, 1], BF16, tag="kfmb")
                cp(out=kfm_sb, in_=kfmp)
                state["kfm"] = kfm_sb
                if aug:
                    # kfb[64*h2+i] = sum_kw kf[64*h2+i, kw] * b_fc[kw]
                    kbp = ppflow.tile([NP_, 1], F32, tag="flow")
                    for j in range(KCH):
                        mm(kbp, kfT_sb[:, j, :], bfcc_sb[:, j:j + 1],
                           start=(j == 0), stop=(j == KCH - 1))
                    kfb_sb = tiny.tile([NP_, 1], BF16, tag="kfb")
                    cp(out=kfb_sb, in_=kbp)
                    state["kfb"] = kfb_sb

            def c_tT():
                sig_sb = state["sig"]
                # tT = Sigma_h @ qT_h for each head (row groups run concurrently)
                qt_sb = work.tile([NP_, N], BF16, tag="qt")
                state["qt"] = qt_sb
                for fs in range(2):
                    tTp = ppflow.tile([NP_, 512], F32, tag="flow")
                    for h2 in range(2):
                        base = h2 * D
                        lhs = sig_sb[base:base + D, base:base + D]
                        mm(tTp[base:base + D, :],
                           lhs, qTa_sb[base:base + D, p, fs * 512:(fs + 1) * 512],
                           start=True, stop=True)
                    nc.vector.tensor_tensor(
                        out=qt_sb[:, fs * 512:(fs + 1) * 512], in0=tTp,
                        in1=qTa_sb[:, p, fs * 512:(fs + 1) * 512],
                        op=mybir.AluOpType.mult)

            def c_stats():
                qt_sb = state["qt"]
                kfm_sb = state["kfm"]
                # mu / sq columns via free-dim-1 matmuls
                ncol = 6 if aug else 4
                msp = ppflow.tile([NP_, 8 * ncol], F32, tag="flow")
                for h2 in range(2):
                    base = h2 * D
                    for j in range(NCH):
                        lq = qTa_sb[base:base + D, p, j * NP_:(j + 1) * NP_]
                        mm(msp[:, (2 * h2 + 0) * 8 + j:(2 * h2 + 0) * 8 + j + 1],
                           lq, kfm_sb[base:base + D, :], start=True, stop=True)
                        mm(msp[:, (2 * h2 + 1) * 8 + j:(2 * h2 + 1) * 8 + j + 1],
                           qt_sb[base:base + D, j * NP_:(j + 1) * NP_],
                           ones_c[base:base + D, :], start=True, stop=True)
                        if aug:
                            mm(msp[:, (4 + h2) * 8 + j:(4 + h2) * 8 + j + 1],
                               lq, state["kfb"][base:base + D, :],
                               start=True, stop=True)
                cp(out=mu_all[:, p, 0:8], in_=msp[:, 0:8])
                cp(out=sq_all[:, p, 0:8], in_=msp[:, 8:16])
                cp(out=mu_all[:, p, 8:16], in_=msp[:, 16:24])
                cp(out=sq_all[:, p, 8:16], in_=msp[:, 24:32])
                if aug:
                    qkfb = work.tile([NP_, 16], F32, tag="qkfb")
                    cp(out=qkfb[:, 0:8], in_=msp[:, 32:40])
                    cp(out=qkfb[:, 8:16], in_=msp[:, 40:48])
                    nc.vector.tensor_scalar(out=qkfb, in0=qkfb, scalar1=2.0,
                                            scalar2=sb2_sb, op0=mybir.AluOpType.mult,
                                            op1=mybir.AluOpType.add)
                    nc.vector.tensor_add(sq_all[:, p, :], sq_all[:, p, :], qkfb)
                    nc.vector.tensor_scalar_add(mu_all[:, p, :], mu_all[:, p, :],
                                                meanb_sb)

            def c_rb():
                # var = sq/KW - mu^2 ; r = rsqrt(var+eps) via Newton on DVE
                # (keeps ScalarE free for the exp stream and avoids any
                # activation-table switching - Exp is the only table used).
                muv = mu_all[:, p, :]
                sqv = sq_all[:, p, :]
                var_t = work.tile([NP_, 16], F32, tag="var")
                nc.vector.tensor_mul(var_t, muv, muv)
                sc_t = work.tile([NP_, 16], F32, tag="sc")
                nc.vector.tensor_scalar(out=sc_t, in0=sqv, scalar1=1.0 / KW,
                                        scalar2=float(EPS),
                                        op0=mybir.AluOpType.mult,
                                        op1=mybir.AluOpType.add)
                nc.vector.tensor_sub(var_t, sc_t, var_t)
                # seed r0 = 2/(1+v) (var is ~[0.4, 2.2]; rel err <= 0.10)
                sd_t = work.tile([NP_, 16], F32, tag="sd")
                nc.vector.tensor_scalar(out=sd_t, in0=var_t, scalar1=0.5,
                                        scalar2=0.5, op0=mybir.AluOpType.mult,
                                        op1=mybir.AluOpType.add)
                r_t = work.tile([NP_, 16], F32, tag="rt")
                nc.vector.reciprocal(r_t, sd_t)
                t1 = work.tile([NP_, 16], F32, tag="t1")
                NIT = 2  # seed err <=0.10 -> 2 Newton iters reach ~1.5e-4
                for it in range(NIT):
                    nc.vector.tensor_mul(t1, r_t, r_t)
                    nc.vector.tensor_mul(t1, t1, var_t)
                    nc.vector.tensor_scalar(out=t1, in0=t1, scalar1=-0.5,
                                            scalar2=1.5,
                                            op0=mybir.AluOpType.mult,
                                            op1=mybir.AluOpType.add)
                    dst = r_all[:, p, :] if it == NIT - 1 else r_t
                    nc.vector.tensor_mul(dst, r_t, t1)
                    # (seed err ~0.1 -> 1.5e-4 after 2 iters; plenty under
                    # the bf16 score noise of ~4e-3)
                nc.vector.scalar_tensor_tensor(out=b_all[:, p, :], in0=muv,
                                               scalar=-1.0, in1=r_all[:, p, :],
                                               op0=mybir.AluOpType.mult,
                                               op1=mybir.AluOpType.mult)

            return [c_kf0, c_kf1, c_tr, c_sig, c_tT, c_stats, c_rb]

        # ---------------- B2: softmax / AV for one (pair, seq-chunk) ----------------
        def emit_scores_exp(p, j):
            # One head at a time: ScalarE exps one [128,1024] score tile while
            # PE fills the other "ap" slot -> the exp stream (the kernel's
            # serial backbone) never waits, and only 2 ap slots are needed.
            h0 = 2 * p
            saved = []
            for h2 in range(2):
                base = h2 * D
                ap_ = ppap.tile([NP_, KW], F32, tag="ap")
                lq = qTa_sb[base:base + D, p, j * NP_:(j + 1) * NP_]
                mm(ap_[:, 0:512], lq, kfa_sb[base:base + D, p, 0:512],
                   start=True, stop=True)
                mm(ap_[:, 512:1024], lq, kfa_sb[base:base + D, p, 512:1024],
                   start=True, stop=True)
                rcol = r_all[:, p, h2 * 8 + j:h2 * 8 + j + 1]
                bcol = b_all[:, p, h2 * 8 + j:h2 * 8 + j + 1]
                e_t = epool.tile([NP_, KW], BF16, tag="e")
                S_t = tiny.tile([NP_, 1], F32, tag="S")
                if not (aug or gb):
                    nc.scalar.activation(e_t, ap_,
                                         mybir.ActivationFunctionType.Exp,
                                         bias=bcol, scale=rcol, accum_out=S_t)
                else:
                    u_t = work.tile([NP_, KW], F32, tag="u")
                    src = ap_
                    if aug:
                        nc.vector.tensor_add(u_t, ap_, bfc_bc)
                        src = u_t
                    if gb:
                        w_t = work.tile([NP_, KW], F32, tag="w")
                        nc.scalar.activation(w_t, src,
                                             mybir.ActivationFunctionType.Identity,
                                             bias=bcol, scale=rcol)
                        nc.vector.tensor_mul(w_t, w_t, gam_bc)
                        nc.vector.tensor_add(w_t, w_t, bet_bc)
                        nc.scalar.activation(e_t, w_t,
                                             mybir.ActivationFunctionType.Exp,
                                             bias=0.0, scale=1.0, accum_out=S_t)
                    else:
                        nc.scalar.activation(e_t, src,
                                             mybir.ActivationFunctionType.Exp,
                                             bias=bcol, scale=rcol, accum_out=S_t)
                saved.append((e_t, S_t))
            return saved

        def emit_av(p, j, yTp, saved):
            # The v/S normalize is emitted HERE (not with the exp) so that
            # the v-projection thunks for this chunk are already emitted:
            # Tile's dependency tracking is emission-order-based, so a read
            # emitted before its writer would order as write-after-read and
            # consume uninitialized SBUF.
            h0 = 2 * p
            for h2 in range(2):
                base = h2 * D
                e_t, S_t = saved[h2]
                h = h0 + h2
                # v/S on the (otherwise idle) GPSIMD engine: keeps the AV
                # gating chain off the busy DVE queue entirely.
                vp_t = tiny.tile([NP_, D], BF16, tag="vp")
                nc.gpsimd.normalize_recip(vp_t, v_sb[:, j, h * D:(h + 1) * D],
                                          S_t)
                mm(yTp[base:base + D, 0:512], vp_t, e_t[:, 0:512],
                   start=(j == 0), stop=(j == NCH - 1), tile_position=(0, base),
                   skip_group_check=True)
                mm(yTp[base:base + D, 512:1024], vp_t, e_t[:, 512:1024],
                   start=(j == 0), stop=(j == NCH - 1), tile_position=(0, base),
                   skip_group_check=True)

        # ---------------- B3: transpose + fc2 + output DMA for one pair ----------------
        # Emitted as three thunks woven into the NEXT pair's B2 loop so the
        # transposes/fc2 never sit ahead of the next scores in the PE queue.
        def b3_thunks(p, yTp):
            state = {}

            def t_tr():
                cp(out=yT_all[:, p, :], in_=yTp)
                y_sb = work.tile([NP_, KCH, NP_], BF16, tag="y")
                state["y"] = y_sb
                for j4 in range(KCH // 4):
                    ytr = ppflow.tile([NP_, 4 * NP_], BF16, tag="flow")
                    for q4 in range(4):
                        jj = 4 * j4 + q4
                        nc.tensor.transpose(
                            ytr[:, q4 * NP_:(q4 + 1) * NP_],
                            yT_all[:, p, jj * NP_:(jj + 1) * NP_], ident)
                    cp(out=y_sb[:, 4 * j4:4 * j4 + 4, :], in_=ytr)
                o2_sb = work.tile([NP_, N], F32, tag="o2")
                state["o2"] = o2_sb

            def t_o2(half, fire_dma):
                def th():
                    y_sb = state["y"]
                    o2_sb = state["o2"]
                    o2p = ppflow.tile([NP_, 512], F32, tag="flow")
                    last = KCH - 1
                    for j in range(KCH):
                        st = (j == 0)
                        sp = (j == last) and not aug2
                        mm(o2p, y_sb[:, j, :],
                           wfc2T_sb[:, j, half * 512:(half + 1) * 512],
                           start=st, stop=sp)
                    if aug2:
                        mm(o2p, ones_r, bfc2_sb[:, half * 512:(half + 1) * 512],
                           start=False, stop=True)
                    cp(out=o2_sb[:, half * 512:(half + 1) * 512], in_=o2p)
                    if fire_dma:
                        nc.sync.dma_start(out=out_d[p], in_=o2_sb)
                return th

            return [t_tr, t_o2(0, False), t_o2(1, True)]

        # ---------------- emission schedule ----------------
        # Startup critical path: narrow k block -> qT(0) -> B1(0) -> r/b(0)
        # -> first scores/exp.  Everything else trickles in as thunks.
        if os.environ.get("K_NARROW_OFF"):
            for n in range(NCH):
                kvp = ppflow.tile([NP_, 512], F32, tag="flow")
                for c in range(CCH):
                    mm(kvp, xT_sb[:, c, n * NP_:(n + 1) * NP_],
                       wkvT_sb[:, c, 0:512],
                       start=(c == 0), stop=(c == CCH - 1))
                nc.scalar.copy(out=kv_sb[:, n, 0:512], in_=kvp)
        else:
            emit_k_narrow()
        # kf(0) first (it chases the wfcT DMA); qT(0) is only needed from
        # the Sigma@qT step onward, by which time wqT has landed.
        b10 = b1_chunks(0)
        b10[0]()
        b10[1]()
        emit_qT(0, scalar_cp=True)
        for th in b10[2:]:
            th()
        # Remaining k columns (pairs 1-3 only need these + fs1).
        for n in range(NCH):
            emit_k_rest(n)

        def _kv_thunk(fs, n):
            return lambda: emit_kv_chunk(fs, n, scalar_cp=False)

        def _qt_thunk(m):
            return lambda: emit_qT(m, scalar_cp=False)

        # Main loop, software-pipelined three ways:
        #  - within a pair: scores/exp for chunk j+1 are emitted BEFORE the
        #    AV matmuls of chunk j, so an AV waiting on its v/S operand
        #    never blocks the next scores in the PE queue;
        #  - across pairs: pair p+1's B1 chunks (pair 0: also the fs1/fs2
        #    projection chunks) are woven between pair p's B2 steps, three
        #    per step, so r/b for p+1 is ready chunks before its first exp;
        #  - B3(p) is woven into pair p+1's loop the same way.
        b3_pending = []
        for p in range(NPAIR):
            thunks = list(b3_pending)
            b3_pending = []
            if p == 0:
                # fs1 chunks carry v cols 0:256 (pairs 0/1) + k tail; B1(1)
                # leads (its r/b chain is long), fs1 woven so chunk j's v is
                # emitted before step j's AV; fs2 (v for pairs 2-5) last.
                b1n = [_qt_thunk(1)] + b1_chunks(1)
                fs1 = [_kv_thunk(1, n) for n in range(NCH)]
                fs2 = [_kv_thunk(2, n) for n in range(NCH)]
                mix = []
                for i in range(NCH):
                    mix.append(fs1[i])
                    if i < len(b1n):
                        mix.append(b1n[i])
                mix.extend(b1n[NCH:])
                thunks += mix + fs2
            elif p + 1 < NPAIR:
                thunks += [_qt_thunk(p + 1)] + b1_chunks(p + 1)
            nxt = iter(thunks)
            yTp = pphold.tile([NP_, KW], F32, tag="hold")
            pend = emit_scores_exp(p, 0)
            for j in range(NCH):
                nxt_pend = emit_scores_exp(p, j + 1) if j + 1 < NCH else None
                # front-load: 3 thunks per step so the next pair's stats
                # (and the serial r/b chain) finish several chunks early
                for _ in range(3):
                    th = next(nxt, None)
                    if th is not None:
                        th()
                emit_av(p, j, yTp, pend)
                pend = nxt_pend
            for th in nxt:
                th()
            if os.environ.get("B3_INLINE"):
                for th in b3_thunks(p, yTp):
                    th()
            else:
                b3_pending = b3_thunks(p, yTp)
        # Tail: keep the HAM clock warm through the dependency-serial fc2
        # tail of the last pair (transposes don't count as PE-busy).
        wupb = ppap.tile([NP_, 512], F32, tag="ap")
        for _ in range(16):
            mm(wupb, warm_sb[:, 0:128], warm_sb, start=True, stop=True)
        for th in b3_pending:
            th()

    nc.compile()
    return nc


def _bf(a):
    return np.ascontiguousarray(a.astype(ml_dtypes.bfloat16))


def kernel(x, w_qkv, w_fc, b_fc, gamma, beta, w_fc2, b_fc2, **_ignore):
    global LAST_RESULT
    x = np.asarray(x, np.float32)
    w_qkv = np.asarray(w_qkv, np.float32)
    w_fc = np.asarray(w_fc, np.float32)
    b_fc = np.asarray(b_fc, np.float32)
    gamma = np.asarray(gamma, np.float32)
    beta = np.asarray(beta, np.float32)
    w_fc2 = np.asarray(w_fc2, np.float32)
    b_fc2 = np.asarray(b_fc2, np.float32)

    aug = bool(np.any(b_fc != 0.0))
    gb = bool(np.any(gamma != 1.0) or np.any(beta != 0.0))
    aug2 = bool(np.any(b_fc2 != 0.0))

    key = (aug, gb, aug2)
    if key not in _CACHE:
        _CACHE[key] = _build(aug, gb, aug2)
    nc = _CACHE[key]

    wq = (w_qkv[0:C] * SCALE).T          # [C, C] columns = q dims
    wkv = w_qkv[C:3 * C].T               # [C, 2C] columns = k dims then v dims
    shared = {
        "wqT": _bf(wq).reshape(CCH, NP_, C),
        "wkvT": _bf(wkv).reshape(CCH, NP_, 2 * C),
        "wfcT": _bf(w_fc.T).reshape(NCH, NP_, KW),
        "wfc2T": _bf(w_fc2.T).reshape(KCH, NP_, N),
    }
    if aug:
        shared["bfc"] = b_fc.reshape(1, KW)
        shared["bfcc"] = _bf(b_fc.reshape(KCH, NP_).T)
        shared["cmeanb"] = np.array([[b_fc.mean()]], np.float32)
        shared["csb2"] = np.array([[(b_fc ** 2).sum()]], np.float32)
    if gb:
        shared["gam"] = gamma.reshape(1, KW).astype(np.float32)
        shared["bet"] = beta.reshape(1, KW).astype(np.float32)
    if aug2:
        shared["bfc2"] = _bf(b_fc2.reshape(1, N))

    in_maps = []
    for b in range(B):
        m = dict(shared)
        m["xT"] = _bf(x[b].T).reshape(CCH, NP_, N)
        in_maps.append(m)

    res = run_bass_kernel_spmd(nc, in_maps, core_ids=list(range(8)))
    LAST_RESULT = res

    out = np.empty((B, N, C), np.float32)
    for b in range(B):
        outT = res.results[b]["out"].reshape(C, N)
        out[b] = outT.T
    return out
